# revision 14
# baseline (speedup 1.0000x reference)
"""Trainium2 kernel for nn_Attn_55516747268530 (LSH bucket attention).

Sharding: one head per NeuronCore (H=8, n_cores=8); each core computes the
per-bucket RBF attention for both hash rounds (L=2) of its head — the
FLOP-dominant stage.  The host prepares the E2LSH bucketing (hash keys,
argsort, gather into sorted order) and applies the inverse permutation +
output projection/FFN on the results.

The kernel is TENSOR-QUEUE bound: without fast-weight-load (walrus rejects
--enable-ldw-opt on bass-emitted LDWEIGHTS), each matmul pays a weight-load
of `stationary_cols/1.2GHz` on the PE queue.  So the structure minimizes
total stationary columns:
  mm1 (arg[k,q] = skT_aug.T @ sqT_aug): stationary skT slice, 128 cols.
  mm2 (oT[d,q]  = sv.T @ dists):        stationary sv, 65 cols (64 v + ones
                                        column that accumulates the denom).
mm2's output is [65(v-dims+den), q] in PSUM; the DVE evicts it to f16 SBUF
(den rides along in f16 — error ~5e-4, well within tolerance) and one DMA
per 16-bucket group stores [65, 2048] to DRAM.

q/k ship as 70-partition tiles (67 hash channels + qs_hi|1 + 1|ks_hi +
1|ks_lo aug; the q-side |q|^2 needs no lo split — a per-q-row scale in exp
cancels in the o/den ratio), v as 65 columns in k-major order.  Each q/k
group load is a 64-line + 6-line DMA pair so descriptors spread across all
16 HW DMA queues.

Device stage, per core / per (l, bucket):
  arg[k,q]  = skT_aug.T @ sqT_aug   (fp16 operands, fp32 PSUM)
  dists     = exp(arg)              (ACT from PSUM; min(arg,0) redundant)
  oT[:, q]  = [v | 1].T @ dists     (row 64 = denominator)
"""
import os
import sys

sys.path.insert(0, "/opt/trn_rl_repo")

import numpy as np

N, H, D, R, K, L, BS = 32768, 8, 64, 3, 8, 2, 128
HASH_DIM = D + R           # 67
CQK = 70                   # q/k partitions: 64 + [coords, qs_hi, 1, 1]/[coords, 1, ks_hi, ks_lo]
AUG = CQK - D              # 6 aug partitions
VP = 65                    # v columns: 64 v + 1 ones (denominator)
NB = N // BS               # 256 buckets
GB = 16                    # buckets per DMA group
SB = 8                     # buckets per mm2 PSUM batch (p2 = [65, 1024] = 2 banks)
SS = 4                     # buckets per mm1/exp batch (p1 = 1 PSUM bank)
N_CORES = 8

_cache = {}
_EXP = None


def _build_nc():
    import concourse.bass as bass
    import concourse.mybir as mybir
    from concourse import bacc, tile

    f32 = mybir.dt.float32
    f16 = mybir.dt.float16
    ts = bass.ts
    nc = bacc.Bacc("TRN2", target_bir_lowering=False, debug=False,
                   num_devices=N_CORES)
    sqT = nc.dram_tensor("sqT", [L, CQK, N], f16, kind="ExternalInput")
    skT = nc.dram_tensor("skT", [L, CQK, N], f16, kind="ExternalInput")
    # k-major: [L, BS(k), NB, VP] so group loads are contiguous per partition
    sv = nc.dram_tensor("sv", [L, BS, NB, VP], f16, kind="ExternalInput")
    # output: [L, 65, N] f16; rows 0:64 = o^T (v-dims), row 64 = denominator
    o = nc.dram_tensor("o", [L, VP, N], f16, kind="ExternalOutput")

    with tile.TileContext(nc) as tc:
        with (tc.tile_pool(name="qk", bufs=8) as qkpool,
              tc.tile_pool(name="v", bufs=6) as vpool,
              tc.tile_pool(name="d", bufs=5) as dpool,
              tc.tile_pool(name="o", bufs=4) as opool,
              tc.tile_pool(name="p1", bufs=4, space="PSUM") as p1pool,
              tc.tile_pool(name="p2", bufs=2, space="PSUM") as p2pool):
            # HAM warm-up: ~4us of back-to-back wide matmuls during the
            # initial DMA ramp trips the PE activity monitor to K=8/8
            # (2.4 GHz); the steady LDW+MM mix alone never does.
            wsrc = vpool.tile([BS, 512], f16, tag="warm")
            nc.vector.memset(wsrc[:], 0.0)
            for w in range(10):
                pw = p1pool.tile([BS, 512], f32, tag="p1")
                nc.tensor.matmul(pw[:], wsrc[:, 0:BS], wsrc[:],
                                 start=True, stop=True)

            # Software-pipelined emission: mm2 of batch s is emitted after
            # the mm1/exp of batches s+1..s+LOOKAHEAD, so the PE keeps
            # issuing matmuls while each batch's exp runs on ACT.
            LOOKAHEAD = 4
            pending = []  # (dexp, tv, to, b0, store) — one entry per SS batch

            def flush_one():
                # one p2 flush covers SB buckets = SB//SS pending entries
                group = [pending.pop(0) for _ in range(SB // SS)]
                p2 = p2pool.tile([VP, SB * BS], f32, tag="p2")
                for e, (dexp_, tv_, to_, b0_, store_) in enumerate(group):
                    for j in range(SS):
                        nc.tensor.matmul(p2[:, ts(e * SS + j, BS)],
                                         tv_[:, b0_ + j, :],
                                         dexp_[:, ts(j, BS)],
                                         start=True, stop=True)
                dexp_, tv_, to_, b0_, store_ = group[0]
                nc.vector.tensor_copy(
                    to_[:, b0_ * BS:(b0_ + SB) * BS], p2[:])
                store_last = group[-1][4]
                if store_last is not None:
                    nc.sync.dma_start(store_last, to_[:])

            for l in range(L):
                for g in range(NB // GB):
                    g0 = g * GB
                    # 64-line + 6-line splits: a 64/128-line DMA spreads its
                    # descriptors across all 16 HW queues; a 70-line one
                    # would not.
                    tq = qkpool.tile([CQK, GB * BS], f16, tag="tq")
                    tk = qkpool.tile([CQK, GB * BS], f16, tag="tk")
                    tv = vpool.tile([BS, GB, VP], f16, tag="tv")
                    cs = slice(g0 * BS, (g0 + GB) * BS)
                    nc.sync.dma_start(tq[0:D, :], sqT[l][0:D, cs])
                    nc.sync.dma_start(tq[D:CQK, :], sqT[l][D:CQK, cs])
                    nc.gpsimd.dma_start(tk[0:D, :], skT[l][0:D, cs])
                    nc.gpsimd.dma_start(tk[D:CQK, :], skT[l][D:CQK, cs])
                    nc.gpsimd.dma_start(tv[:], sv[l][:, g0:g0 + GB, :])
                    to = opool.tile([VP, GB * BS], f16, tag="to")
                    for ss in range(GB // SS):
                        s0 = ss * SS
                        p1 = p1pool.tile([BS, SS * BS], f32, tag="p1")
                        for j in range(SS):
                            nc.tensor.matmul(p1[:, ts(j, BS)],
                                             tk[:, ts(s0 + j, BS)],
                                             tq[:, ts(s0 + j, BS)],
                                             start=True, stop=True)
                        dexp = dpool.tile([BS, SS * BS], f16, tag="dexp")
                        nc.scalar.activation(dexp[:], p1[:], _EXP)
                        last_g = s0 + SS == GB
                        pending.append(
                            (dexp, tv, to, s0, o[l][:, cs] if last_g else None))
                        if len(pending) > LOOKAHEAD:
                            flush_one()
            while pending:
                flush_one()
    nc.compile()
    return nc


def _install_ntff_shim():
    """Register the NTFF profile hook missing from this image's antenv stub."""
    import types
    try:
        import antenv.axon_hooks  # noqa: F401
        return
    except ImportError:
        pass
    try:
        import antenv
        from trn_agent_boot.trn_boot import _ntff_profile_via_ctypes
        mod = types.ModuleType("antenv.axon_hooks")
        mod._hook = _ntff_profile_via_ctypes("/opt/axon/libaxon_pjrt.so")
        mod.set_axon_ntff_profile_hook = lambda h: setattr(mod, "_hook", h)
        mod.get_axon_ntff_profile_hook = lambda: mod._hook
        sys.modules["antenv.axon_hooks"] = mod
        antenv.axon_hooks = mod
    except Exception:
        pass


def _device_attention(in_maps, trace=False):
    global _EXP
    from concourse.bass_utils import run_bass_kernel_spmd
    if trace:
        _install_ntff_shim()
    if "nc" not in _cache:
        import concourse.mybir as mybir
        _EXP = mybir.ActivationFunctionType.Exp
        _cache["nc"] = _build_nc()
    nc = _cache["nc"]
    res = run_bass_kernel_spmd(nc, in_maps, list(range(N_CORES)), trace=trace)
    if trace and res.exec_time_ns is not None:
        _cache["exec_time_ns"] = res.exec_time_ns
    return res.results


def kernel(x, coords, combined_shifts, wq, wk, wv, out_w, out_b,
           norm1_g, norm1_b, norm2_g, norm2_b,
           ff1_w, ff1_b, ff2_w, ff2_b, w_rpe_w, alpha):
    f32, f16 = np.float32, np.float16
    x = np.asarray(x, f32)
    coords = np.asarray(coords, f32)
    combined_shifts = np.asarray(combined_shifts)

    # ---- host: layernorm + qkv + hash keys + argsort (plumbing for device) --
    mu = x.mean(-1, keepdims=True, dtype=f32)
    var = ((x - mu) ** 2).mean(-1, keepdims=True, dtype=f32)
    xn = (x - mu) / np.sqrt(var + f32(1e-5)) * norm1_g + norm1_b
    q = (xn @ wq).reshape(N, H, D)
    k = (xn @ wk).reshape(N, H, D)
    v = (xn @ wv).reshape(N, H, D)
    w4 = w_rpe_w.reshape(H, D, R, K)
    qw = np.exp(np.minimum(w4.sum(1), f32(50.0))).sum(-1)
    sqrt_w_r = np.sqrt(f32(2.0) * qw).astype(f32)[None] * coords[:, None, :]
    q_hat = np.concatenate([q, sqrt_w_r], -1).transpose(1, 0, 2)  # (H,N,67)
    k_hat = np.concatenate([k, sqrt_w_r], -1).transpose(1, 0, 2)
    v_t = v.transpose(1, 0, 2)                                    # (H,N,64)

    qh = np.einsum("hnd,hdl->lhn", q_hat, alpha).astype(f32)
    kh = np.einsum("hnd,hdl->lhn", k_hat, alpha).astype(f32)
    hash_shift = (np.maximum(qh.max(-1, keepdims=True), kh.max(-1, keepdims=True))
                  - np.minimum(qh.min(-1, keepdims=True), kh.min(-1, keepdims=True)))
    cs = combined_shifts.astype(f32) * hash_shift
    q_pos = np.argsort(qh + cs, axis=-1, kind="stable")           # (L,H,N)
    k_pos = np.argsort(kh + cs, axis=-1, kind="stable")

    qsq = (f32(-0.5) * (q_hat ** 2).sum(-1)).astype(f32)          # (H,N)
    ksq = (f32(-0.5) * (k_hat ** 2).sum(-1)).astype(f32)

    in_maps = []
    for h in range(N_CORES):
        sqT = np.zeros((L, CQK, N), f16)
        skT = np.zeros((L, CQK, N), f16)
        sv = np.zeros((L, BS, NB, VP), f16)
        for l in range(L):
            qp, kp = q_pos[l, h], k_pos[l, h]
            # channels: 0:67 hash dims; 67: qs_hi|1; 68: 1|ks_hi; 69: 1|ks_lo
            sqT[l, :HASH_DIM] = q_hat[h][qp].T.astype(f16)
            sqT[l, HASH_DIM] = qsq[h][qp].astype(f16)
            sqT[l, HASH_DIM + 1] = 1.0
            sqT[l, HASH_DIM + 2] = 1.0
            skT[l, :HASH_DIM] = k_hat[h][kp].T.astype(f16)
            skT[l, HASH_DIM] = 1.0
            ks = ksq[h][kp]
            ks_hi = ks.astype(f16)
            skT[l, HASH_DIM + 1] = ks_hi
            skT[l, HASH_DIM + 2] = (ks - ks_hi.astype(f32)).astype(f16)
            svl = np.zeros((N, VP), f16)
            svl[:, :D] = v_t[h][kp].astype(f16)
            svl[:, D] = 1.0
            sv[l] = svl.reshape(NB, BS, VP).transpose(1, 0, 2)  # k-major
        in_maps.append({"sqT": sqT, "skT": skT, "sv": sv})

    outs = _device_attention(in_maps, trace=bool(os.environ.get("KERNEL_TRACE")))

    # ---- host: unsort, combine hashes, output projection + FFN -------------
    o_sum = np.zeros((N, H, D), f32)
    d_sum = np.zeros((N, H, 1), f32)
    for h in range(N_CORES):
        ro = outs[h]["o"]      # (L, VP, N) f16: rows 0:64 = o^T, row 64 = den
        for l in range(L):
            qp = q_pos[l, h]
            o_sum[qp, h, :] += ro[l, 0:D].T.astype(f32)
            d_sum[qp, h, 0] += ro[l, D].astype(f32) + f32(1e-20)
    out = (o_sum / d_sum).reshape(N, H * D)

    aggr = out @ out_w + out_b
    x1 = x + aggr
    mu2 = x1.mean(-1, keepdims=True, dtype=f32)
    var2 = ((x1 - mu2) ** 2).mean(-1, keepdims=True, dtype=f32)
    x2 = (x1 - mu2) / np.sqrt(var2 + f32(1e-5)) * norm2_g + norm2_b
    h1 = x2 @ ff1_w + ff1_b
    ff = (h1 / (1 + np.exp(-h1))) @ ff2_w + ff2_b
    return (x1 + ff).astype(f32)
